# revision 32
# baseline (speedup 1.0000x reference)
"""Trainium2 Bass kernel for causal multi-head attention with RoPE.

Reference computation (B=2, T=2048, D=2048, H=16, dk=128):
    Q = x @ Wq.T ; K = x @ Wk.T ; V = x @ Wv.T          (per-head split)
    Q, K <- RoPE(Q, K)
    attn = softmax(mask(Q K^T / sqrt(dk)))
    out  = (attn @ V) merged-heads @ Wo.T
    mask = causal

Sharding (Megatron-style tensor parallel over heads): each of the 8 cores
owns 2 heads (both batches).  Wq/Wk/Wv are sharded column-wise (rows of the
transposed weight), Wo row-wise.  Each core computes a full-shape partial
y^T and the host sums the 8 partials (the all-reduce after Wo).

Device layout choices:
  - x is fed pre-transposed (xT: [D, B*T]) so projections produce Q^T/K^T
    with head-dim on partitions -- the layout QK^T wants.
  - V is produced in natural token-major layout directly by swapping the
    matmul operand roles, with a ones column appended per head so the
    P@[V|1] matmul yields both the attention numerator and the softmax
    denominator in one accumulation (no separate denominator matmul).
  - scores are computed transposed ([keys, queries]); exp(scores) tiles are
    used as the STATIONARY operand of the P@V matmul, so each [128-queries]
    chunk accumulates [q, dk|l] in PSUM; a cheap PE transpose (128 cols per
    query block) restores the [dk, q] layout the output projection needs.
  - no max-subtraction in softmax: scaled scores are ~N(0,1), exp is safe
    in fp32 by a huge margin and matches the reference mathematically.
  - causal masking: off-diagonal key tiles are skipped entirely; the 4
    distinct diagonal-block patterns are multiplicative 0/1 bf16 masks
    applied to exp(scores).
  - RoPE rotate-every-two runs on the DVE with strided even/odd views
    (no PE work); cos/sin muls fused in.
All matmuls run in bf16 (1 cycle/row on the PE vs 4 for fp32).
"""

import sys

sys.path.insert(0, "/opt/trn_rl_repo")

import numpy as np
import ml_dtypes

import concourse.bass as bass  # noqa: F401  (registers engine classes)
import concourse.mybir as mybir
import concourse.tile as tile
from concourse import bacc
from concourse.bass_utils import run_bass_kernel_spmd

BF16 = ml_dtypes.bfloat16

B, T, D, H = 2, 2048, 2048, 16
DK = D // H          # 128
THETA = 10000.0
NCORES = 8
HL = H // NCORES     # 2 local heads per core
DLOC = HL * DK       # 256 local output dims per projection
TOK = B * T          # 4096
P = 128
KD = D // P          # 16 contraction tiles
NT = TOK // 512      # 8 token tiles of 512
QT_PER_B = T // 512  # 4 query tiles per batch
SCALE = 1.0 / float(np.sqrt(DK))

_dt = mybir.dt


def _build_kernel():
    nc = bacc.Bacc("TRN2", target_bir_lowering=False, debug=False,
                   num_devices=NCORES)

    xT = nc.dram_tensor("xT", [D, TOK], _dt.bfloat16, kind="ExternalInput")
    WqT = nc.dram_tensor("WqT", [D, DLOC], _dt.bfloat16, kind="ExternalInput")
    WkT = nc.dram_tensor("WkT", [D, DLOC], _dt.bfloat16, kind="ExternalInput")
    WvT = nc.dram_tensor("WvT", [D, DLOC], _dt.bfloat16, kind="ExternalInput")
    WoT = nc.dram_tensor("WoT", [DLOC, D], _dt.bfloat16, kind="ExternalInput")
    COS = nc.dram_tensor("COS", [P, T], _dt.bfloat16, kind="ExternalInput")
    SIN = nc.dram_tensor("SIN", [P, T], _dt.bfloat16, kind="ExternalInput")
    IDT = nc.dram_tensor("IDT", [P, P], _dt.bfloat16, kind="ExternalInput")
    ROT = nc.dram_tensor("ROT", [P, P], _dt.bfloat16, kind="ExternalInput")
    MD = nc.dram_tensor("MD", [P, 4, 512], _dt.bfloat16, kind="ExternalInput")
    # bf16 partials: halves the output DMA; host accumulates in fp32
    yT = nc.dram_tensor("yT", [D, TOK], _dt.bfloat16, kind="ExternalOutput")

    xT_r = xT.ap().rearrange("(ko p) m -> p ko m", p=P)    # [128, 16, 4096]
    wq_r = WqT.ap().rearrange("(ko p) n -> p ko n", p=P)   # [128, 16, 256]
    wk_r = WkT.ap().rearrange("(ko p) n -> p ko n", p=P)
    wv_r = WvT.ap().rearrange("(ko p) n -> p ko n", p=P)
    wo_r = WoT.ap().rearrange("(ho p) n -> p ho n", p=P)   # [128, 2, 2048]

    with tile.TileContext(nc) as tc:
        with (
            tc.tile_pool(name="const", bufs=1) as cp,
            tc.tile_pool(name="data", bufs=1) as dp,
            tc.tile_pool(name="xs", bufs=2) as xp,
            tc.tile_pool(name="work", bufs=3) as wp,
        ):
            wq_sb = cp.tile([P, KD, DLOC], _dt.bfloat16, tag="wq")
            wk_sb = cp.tile([P, KD, DLOC], _dt.bfloat16, tag="wk")
            wv_sb = cp.tile([P, KD, DLOC], _dt.bfloat16, tag="wv")
            wo_sb = cp.tile([P, HL, D], _dt.bfloat16, tag="wo")
            cos_sb = cp.tile([P, T], _dt.bfloat16, tag="cos")
            sin_sb = cp.tile([P, T], _dt.bfloat16, tag="sin")
            idt_sb = cp.tile([P, P], _dt.bfloat16, tag="idt")
            rot_sb = cp.tile([P, P], _dt.bfloat16, tag="rot")
            md_sb = cp.tile([P, 4, 512], _dt.bfloat16, tag="md")

            # persistent activations (partition = head-dim except v_sb);
            # RoPE is applied in place.
            qt_sb = dp.tile([P, HL, TOK], _dt.bfloat16, tag="qt")
            kt_sb = dp.tile([P, HL, TOK], _dt.bfloat16, tag="kt")
            # V in token-major layout with a ones column per head:
            # [128 tokens, token-tile, head, dk | 1]
            v_sb = dp.tile([P, TOK // P, HL, DK + 1], _dt.bfloat16, tag="v")

            # ones column for the fused denominator (written once)
            nc.vector.memset(v_sb[:, :, :, DK], 1.0)

            # ------- phase A: QKV projections with RoPE interleaved -------
            with tc.tile_pool(name="psproj", bufs=1, space="PSUM") as pp, \
                 tc.tile_pool(name="psv", bufs=2, space="PSUM") as pv:
                # batch-1 token tiles first so phase B (which starts with
                # batch 1) never waits on the phase-A tail
                for idx, nt in enumerate([4, 5, 6, 7, 0, 1, 2, 3]):
                    ts0 = nt * 512
                    # one batched 2MB DMA per token tile (HWDGE cost is
                    # dominated by per-instruction overhead)
                    xts = xp.tile([P, KD, 512], _dt.bfloat16, tag="xt")
                    if idx == 0:
                        # chunked first tile + interleaved one-time weight
                        # loads so the first matmuls start within a few us
                        for kc in [(0, 1), (1, 2), (2, 4)] + \
                                  [(k, k + 4) for k in range(4, KD, 4)]:
                            a, z = kc
                            nc.sync.dma_start(xts[:, a:z, :],
                                              xT_r[:, a:z, ts0:ts0 + 512])
                            nc.sync.dma_start(wq_sb[:, a:z, :],
                                              wq_r[:, a:z, :])
                            nc.sync.dma_start(wk_sb[:, a:z, :],
                                              wk_r[:, a:z, :])
                            nc.sync.dma_start(wv_sb[:, a:z, :],
                                              wv_r[:, a:z, :])
                        # must be emitted before their first readers (the
                        # first RoPE) -- dep tracking is program-order
                        nc.sync.dma_start(cos_sb[:], COS[:])
                        nc.sync.dma_start(sin_sb[:], SIN[:])
                        nc.sync.dma_start(idt_sb[:], IDT[:])
                        nc.sync.dma_start(rot_sb[:], ROT[:])
                    else:
                        nc.sync.dma_start(xts[:], xT_r[:, :, ts0:ts0 + 512])
                        if idx == 1:
                            nc.sync.dma_start(md_sb[:], MD[:])
                            nc.sync.dma_start(wo_sb[:], wo_r)
                    psQ = pp.tile([P, HL, 512], _dt.float32, tag="psQ")
                    psK = pp.tile([P, HL, 512], _dt.float32, tag="psK")
                    for k in range(KD):
                        st = (k == 0)
                        sp = (k == KD - 1)
                        for m in range(HL):
                            nc.tensor.matmul(psQ[:, m, :],
                                             wq_sb[:, k, m * P:(m + 1) * P],
                                             xts[:, k, :], start=st, stop=sp)
                            nc.tensor.matmul(psK[:, m, :],
                                             wk_sb[:, k, m * P:(m + 1) * P],
                                             xts[:, k, :], start=st, stop=sp)
                    for m in range(HL):
                        nc.scalar.copy(qt_sb[:, m, ts0:ts0 + 512],
                                       psQ[:, m, :])
                        nc.scalar.copy(kt_sb[:, m, ts0:ts0 + 512],
                                       psK[:, m, :])

                    # V in natural layout: one PSUM bank per token block.
                    # The RoPE rot matmuls (which mix adjacent PARTITIONS,
                    # so they need the PE) are sandwiched between the V
                    # halves so the RoPE DVE chain of the final tile drains
                    # under PE work instead of stalling the phase switch.
                    def emit_v(tb):
                        psv = pv.tile([P, DLOC], _dt.float32, tag="psV",
                                      name=f"psv_{nt}_{tb}")
                        for k in range(KD):
                            nc.tensor.matmul(psv[:],
                                             xts[:, k, tb * P:(tb + 1) * P],
                                             wv_sb[:, k, :],
                                             start=(k == 0), stop=(k == KD - 1))
                        for m in range(HL):
                            nc.scalar.copy(
                                v_sb[:, nt * 4 + tb, m, 0:DK],
                                psv[:, m * P:(m + 1) * P])

                    emit_v(0)
                    emit_v(1)
                    c0 = (nt % QT_PER_B) * 512
                    for src in (qt_sb, kt_sb):
                        for m in range(HL):
                            rp = pv.tile([P, 512], _dt.float32, tag="rot")
                            nc.tensor.matmul(rp[:], rot_sb[:],
                                             src[:, m, ts0:ts0 + 512],
                                             start=True, stop=True)
                            t1 = wp.tile([P, 512], _dt.float32, tag="t1")
                            nc.vector.tensor_mul(t1[:],
                                                 src[:, m, ts0:ts0 + 512],
                                                 cos_sb[:, c0:c0 + 512])
                            t2 = wp.tile([P, 512], _dt.float32, tag="t2")
                            nc.vector.tensor_mul(t2[:], rp[:],
                                                 sin_sb[:, c0:c0 + 512])
                            nc.vector.tensor_add(src[:, m, ts0:ts0 + 512],
                                                 t1[:], t2[:])
                    emit_v(2)
                    emit_v(3)

            # ------- phase B: attention with output proj interleaved -------
            # scores [keys, queries]; exp tiles pT become the STATIONARY
            # operand of P@[V|1], giving op[qc] = [128q, dk | l] per query
            # chunk; normalize on DVE, transpose back on PE.
            # Phase-B software pipeline: each (b,qt) group's output
            # projection is deferred and spliced as PE filler into the NEXT
            # group's exp-paced QK loop, so the in-order PE queue never
            # starves while the ACT engine computes exps.
            with tc.tile_pool(name="psatt", bufs=1, space="PSUM") as pa, \
                 tc.tile_pool(name="psy", bufs=2, space="PSUM") as py:
                filler = []

                def pop_filler(n):
                    for _ in range(min(n, len(filler))):
                        filler.pop(0)()

                for b in (1, 0):
                    # smallest tile first (it gets no filler work), then
                    # descending so later groups keep the filler pipe full
                    for qt in ([0, 3, 2, 1] if b == 1 else [3, 2, 1, 0]):
                        q0 = b * T + qt * 512
                        nk = (qt + 1) * 4
                        otT_sbs = []
                        pend_T = [None]

                        def emit_T(hl, ot_sb, out_list, b_=b, qt_=qt):
                            # lives in the "op" tag rotation (same banks)
                            otT_ps = pa.tile([P, 512], _dt.bfloat16,
                                             tag="op", bufs=2,
                                             padded_shape=[P, 1024],
                                             name=f"oT_{b_}_{qt_}_{hl}")
                            for qc in range(4):
                                nc.tensor.transpose(
                                    otT_ps[:, qc * P:(qc + 1) * P],
                                    ot_sb[:, qc, :], idt_sb)
                            otT = wp.tile([P, 512], _dt.bfloat16, tag="otTs",
                                          bufs=4, name=f"oTs_{b_}_{qt_}_{hl}")
                            nc.vector.tensor_copy(otT, otT_ps)
                            out_list.append(otT)

                        for hl in range(HL):
                            ops = {}
                            ot_sb = wp.tile([P, 4, P], _dt.bfloat16,
                                            tag="ot", bufs=2,
                                            name=f"ot_{b}_{qt}_{hl}")

                            # kt pairs processed DIAGONAL-FIRST: the diag
                            # tiles' exp->mask(DVE)->PV chain warms up at
                            # the loop start; the mask-free off-diagonal
                            # tiles close out the loop with no DVE dep.
                            order = list(range(4 * qt, nk)) + \
                                list(range(4 * qt))
                            prs = [(order[2 * i], order[2 * i + 1])
                                   for i in range(nk // 2)]
                            pos = {kt: i for i, kt in enumerate(order)}
                            pTs = {}

                            def emit_qk_pair(pr):
                                # two key tiles share one PSUM pair tile and
                                # ONE exp instruction (halves ACT overhead);
                                # the unwritten tail of diagonal tiles is
                                # exp'd too but never read downstream.
                                sp_ = pa.tile([P, 2, 512], _dt.float32,
                                              tag="s", bufs=2,
                                              name=f"s_{b}_{qt}_{hl}_{pr}")
                                pT = wp.tile([P, 2, 512], _dt.bfloat16,
                                             tag="pT", bufs=10,
                                             name=f"p_{b}_{qt}_{hl}_{pr}")
                                for half, kt in enumerate(prs[pr]):
                                    j = kt - 4 * qt
                                    qoff = max(j, 0) * P
                                    nq = 512 - qoff
                                    k0 = b * T + kt * P
                                    nc.tensor.matmul(
                                        sp_[:, half, :nq],
                                        kt_sb[:, hl, k0:k0 + P],
                                        qt_sb[:, hl, q0 + qoff:q0 + 512],
                                        start=True, stop=True)
                                nc.scalar.activation(
                                    pT[:], sp_[:],
                                    mybir.ActivationFunctionType.Exp,
                                    scale=SCALE)
                                for half, kt in enumerate(prs[pr]):
                                    j = kt - 4 * qt
                                    qoff = max(j, 0) * P
                                    if j >= 0:  # 0/1 mask in the diagonal
                                        nc.vector.tensor_mul(
                                            pT[:, half, :512 - qoff],
                                            pT[:, half, :512 - qoff],
                                            md_sb[:, j, qoff:])
                                    pTs[kt] = (pT, half, qoff)

                            def emit_pv(kt, qcs):
                                pT, half, qoff = pTs[kt]
                                for qc in qcs:
                                    if kt > 4 * qt + qc:
                                        continue  # fully masked
                                    valid = [k for k in order
                                             if k <= 4 * qt + qc]
                                    st = (kt == valid[0])
                                    sp2 = (kt == valid[-1])
                                    if st:
                                        ops[qc] = pa.tile(
                                            [P, DK + 1], _dt.float32,
                                            tag="op", bufs=2,
                                            padded_shape=[P, 512],
                                            name=f"op_{b}_{qt}_{hl}_{qc}")
                                    c0_ = qc * P - qoff
                                    nc.tensor.matmul(
                                        ops[qc][:, :],
                                        pT[:, half, c0_:c0_ + P],
                                        v_sb[:, b * (T // P) + kt, hl, :],
                                        start=st, stop=sp2)
                                    if sp2:
                                        rec = wp.tile(
                                            [P, 1], _dt.float32, tag="rec",
                                            bufs=8,
                                            name=f"rc_{b}_{qt}_{hl}_{qc}")
                                        nc.vector.reciprocal(
                                            rec, ops[qc][:, DK:DK + 1])
                                        nc.vector.tensor_scalar_mul(
                                            ot_sb[:, qc, :],
                                            ops[qc][:, 0:DK], rec)

                            # pipeline: QK/exp one pair ahead of pass-1
                            # PVs; filler plugs the exp-paced PE slots;
                            # the previous head's transposes are spliced
                            # in after this head's first pair.
                            npr = nk // 2
                            emit_qk_pair(0)
                            for pr in range(1, npr):
                                emit_qk_pair(pr)
                                if pr == 1 and pend_T[0] is not None:
                                    pend_T[0]()
                                    pend_T[0] = None
                                emit_pv(prs[pr - 1][0], (0, 1))
                                emit_pv(prs[pr - 1][1], (0, 1))
                                pop_filler(1)
                            if pend_T[0] is not None:
                                pend_T[0]()
                                pend_T[0] = None
                            emit_pv(prs[npr - 1][0], (0, 1))
                            emit_pv(prs[npr - 1][1], (0, 1))
                            pop_filler(3)
                            for kt in order:
                                emit_pv(kt, (2, 3))
                            pop_filler(2)
                            pend_T[0] = (lambda hl_=hl, ot_=ot_sb,
                                         f_=emit_T, lst_=otT_sbs:
                                         f_(hl_, ot_, lst_))

                        # flush any remaining previous-group output work,
                        # then defer THIS group's output projection as
                        # filler for the next group's exp-paced QK loop.
                        pop_filler(len(filler))
                        last_grp = (b == 0 and qt == 0)
                        filler.append(pend_T[0])
                        pend_T[0] = None
                        state = {}

                        def mk_nb(nb, b_=b, qt_=qt, q0_=q0, otl=otT_sbs,
                                  st_=state, last=last_grp):
                            def go():
                                i = nb % 4
                                if i == 0:
                                    if nb > 0:
                                        nc.sync.dma_start(
                                            yT[(nb - 4) * P:nb * P,
                                               q0_:q0_ + 512]
                                            .rearrange("(i p) q -> p i q",
                                                       p=P), st_["ysb"])
                                    st_["ysb"] = wp.tile(
                                        [P, 4, 512], _dt.bfloat16,
                                        tag="ysb", bufs=3,
                                        name=f"ysb_{b_}_{qt_}_{nb}")
                                yp = py.tile([P, 512], _dt.float32, tag="y",
                                             name=f"y_{b_}_{qt_}_{nb}")
                                for hl in range(HL):
                                    nc.tensor.matmul(
                                        yp,
                                        wo_sb[:, hl, nb * P:(nb + 1) * P],
                                        otl[hl],
                                        start=(hl == 0), stop=(hl == HL - 1))
                                nc.vector.tensor_copy(
                                    st_["ysb"][:, i, :], yp)
                                if last and nb in (13, 15):
                                    # halve the final DMAs so the kernel
                                    # tail is a 2-row transfer, not four
                                    nc.sync.dma_start(
                                        yT[(nb - 1) * P:(nb + 1) * P,
                                           q0_:q0_ + 512]
                                        .rearrange("(i p) q -> p i q", p=P),
                                        st_["ysb"][:, i - 1:i + 1, :])
                                elif nb == 15 and not last:
                                    nc.sync.dma_start(
                                        yT[12 * P:16 * P, q0_:q0_ + 512]
                                        .rearrange("(i p) q -> p i q", p=P),
                                        st_["ysb"])
                            return go

                        for nb in range(16):
                            filler.append(mk_nb(nb))
                # drain the last group's deferred output projection
                pop_filler(len(filler))

    nc.compile()
    return nc


_NC_CACHE = None


def _get_nc():
    global _NC_CACHE
    if _NC_CACHE is None:
        _NC_CACHE = _build_kernel()
    return _NC_CACHE


def _rope_tables():
    inv_freq = 1.0 / THETA ** (np.arange(0, DK, 2, dtype=np.float32) / DK)
    t = np.arange(T, dtype=np.float32)
    freqs = np.outer(t, inv_freq)                 # (T, dk/2)
    freqs = np.repeat(freqs, 2, axis=-1)          # (T, dk)
    return np.cos(freqs), np.sin(freqs)


def _host_inputs(x, Wq, Wk, Wv, Wo):
    """Build the per-core input maps (all host-side prep is free)."""
    xT = np.ascontiguousarray(
        x.reshape(TOK, D).T).astype(BF16)          # [D, B*T]
    cos, sin = _rope_tables()                      # (T, dk)
    cosT = np.ascontiguousarray(cos.T).astype(BF16)  # [128, T]
    sinT = np.ascontiguousarray(sin.T).astype(BF16)

    idt = np.eye(P, dtype=np.float32).astype(BF16)

    rot = np.zeros((P, P), dtype=np.float32)
    for i in range(P // 2):
        rot[2 * i + 1, 2 * i] = -1.0   # (R^T)[2i, 2i+1] = -1
        rot[2 * i, 2 * i + 1] = 1.0    # (R^T)[2i+1, 2i] = +1
    rot = rot.astype(BF16)

    # diagonal-block masks, scores layout [key, query]; offset j*128
    md = np.zeros((4, P, 512), dtype=np.float32)
    kk = np.arange(P)[:, None]
    qq = np.arange(512)[None, :]
    for j in range(4):
        md[j] = (qq >= kk + j * P).astype(np.float32)
    md = np.ascontiguousarray(md.transpose(1, 0, 2)).astype(BF16)

    in_maps = []
    for c in range(NCORES):
        rows = slice(c * DLOC, (c + 1) * DLOC)
        in_maps.append({
            "xT": xT,
            "WqT": np.ascontiguousarray(Wq[rows, :].T).astype(BF16),
            "WkT": np.ascontiguousarray(Wk[rows, :].T).astype(BF16),
            "WvT": np.ascontiguousarray(Wv[rows, :].T).astype(BF16),
            "WoT": np.ascontiguousarray(Wo[:, rows].T).astype(BF16),
            "COS": cosT, "SIN": sinT, "IDT": idt, "ROT": rot, "MD": md,
        })
    return in_maps


def _run(in_maps, **kwargs):
    nc = _get_nc()
    return run_bass_kernel_spmd(nc, in_maps, core_ids=list(range(NCORES)),
                                **kwargs)


def kernel(x, Wq, Wk, Wv, Wo, mask, _bench_results=None, **_kw):
    x = np.asarray(x, dtype=np.float32)
    Wq = np.asarray(Wq, dtype=np.float32)
    Wk = np.asarray(Wk, dtype=np.float32)
    Wv = np.asarray(Wv, dtype=np.float32)
    Wo = np.asarray(Wo, dtype=np.float32)
    mask = np.asarray(mask)
    causal = np.array_equal(mask.reshape(T, T),
                            np.tril(np.ones((T, T), dtype=bool)))
    if not causal:
        raise NotImplementedError("kernel specialized for the causal mask")

    res = _run(_host_inputs(x, Wq, Wk, Wv, Wo))
    if _bench_results is not None:
        _bench_results.append(res)

    acc = np.zeros((D, TOK), dtype=np.float32)
    for r in res.results:
        acc += r["yT"].astype(np.float32)
    # yT[n, b*T + t] -> out[b, t, n]
    return np.ascontiguousarray(acc.reshape(D, B, T).transpose(1, 2, 0))


# revision 33
# speedup vs baseline: 1.0000x; 1.0000x over previous
"""Trainium2 Bass kernel for causal multi-head attention with RoPE.

Reference computation (B=2, T=2048, D=2048, H=16, dk=128):
    Q = x @ Wq.T ; K = x @ Wk.T ; V = x @ Wv.T          (per-head split)
    Q, K <- RoPE(Q, K)
    attn = softmax(mask(Q K^T / sqrt(dk)))
    out  = (attn @ V) merged-heads @ Wo.T
    mask = causal

Sharding (Megatron-style tensor parallel over heads): each of the 8 cores
owns 2 heads (both batches).  Wq/Wk/Wv are sharded column-wise (rows of the
transposed weight), Wo row-wise.  Each core computes a full-shape partial
y^T and the host sums the 8 partials (the all-reduce after Wo).

Device layout choices:
  - x is fed pre-transposed (xT: [D, B*T]) so projections produce Q^T/K^T
    with head-dim on partitions -- the layout QK^T wants.
  - V is produced in natural token-major layout directly by swapping the
    matmul operand roles, with a ones column appended per head so the
    P@[V|1] matmul yields both the attention numerator and the softmax
    denominator in one accumulation (no separate denominator matmul).
  - scores are computed transposed ([keys, queries]); exp(scores) tiles are
    used as the STATIONARY operand of the P@V matmul, so each [128-queries]
    chunk accumulates [q, dk|l] in PSUM; a cheap PE transpose (128 cols per
    query block) restores the [dk, q] layout the output projection needs.
  - no max-subtraction in softmax: scaled scores are ~N(0,1), exp is safe
    in fp32 by a huge margin and matches the reference mathematically.
  - causal masking: off-diagonal key tiles are skipped entirely; the 4
    distinct diagonal-block patterns are multiplicative 0/1 bf16 masks
    applied to exp(scores).
  - RoPE rotate-every-two runs on the DVE with strided even/odd views
    (no PE work); cos/sin muls fused in.
All matmuls run in bf16 (1 cycle/row on the PE vs 4 for fp32).
"""

import sys

sys.path.insert(0, "/opt/trn_rl_repo")

import numpy as np
import ml_dtypes

import concourse.bass as bass  # noqa: F401  (registers engine classes)
import concourse.mybir as mybir
import concourse.tile as tile
from concourse import bacc
from concourse.bass_utils import run_bass_kernel_spmd

BF16 = ml_dtypes.bfloat16

B, T, D, H = 2, 2048, 2048, 16
DK = D // H          # 128
THETA = 10000.0
NCORES = 8
HL = H // NCORES     # 2 local heads per core
DLOC = HL * DK       # 256 local output dims per projection
TOK = B * T          # 4096
P = 128
KD = D // P          # 16 contraction tiles
NT = TOK // 512      # 8 token tiles of 512
QT_PER_B = T // 512  # 4 query tiles per batch
SCALE = 1.0 / float(np.sqrt(DK))

_dt = mybir.dt


def _build_kernel():
    nc = bacc.Bacc("TRN2", target_bir_lowering=False, debug=False,
                   num_devices=NCORES)

    xT = nc.dram_tensor("xT", [D, TOK], _dt.bfloat16, kind="ExternalInput")
    WqT = nc.dram_tensor("WqT", [D, DLOC], _dt.bfloat16, kind="ExternalInput")
    WkT = nc.dram_tensor("WkT", [D, DLOC], _dt.bfloat16, kind="ExternalInput")
    WvT = nc.dram_tensor("WvT", [D, DLOC], _dt.bfloat16, kind="ExternalInput")
    WoT = nc.dram_tensor("WoT", [DLOC, D], _dt.bfloat16, kind="ExternalInput")
    COS = nc.dram_tensor("COS", [P, T], _dt.bfloat16, kind="ExternalInput")
    SIN = nc.dram_tensor("SIN", [P, T], _dt.bfloat16, kind="ExternalInput")
    IDT = nc.dram_tensor("IDT", [P, P], _dt.bfloat16, kind="ExternalInput")
    ROT = nc.dram_tensor("ROT", [P, P], _dt.bfloat16, kind="ExternalInput")
    MD = nc.dram_tensor("MD", [P, 4, 512], _dt.bfloat16, kind="ExternalInput")
    # bf16 partials: halves the output DMA; host accumulates in fp32
    yT = nc.dram_tensor("yT", [D, TOK], _dt.bfloat16, kind="ExternalOutput")

    xT_r = xT.ap().rearrange("(ko p) m -> p ko m", p=P)    # [128, 16, 4096]
    wq_r = WqT.ap().rearrange("(ko p) n -> p ko n", p=P)   # [128, 16, 256]
    wk_r = WkT.ap().rearrange("(ko p) n -> p ko n", p=P)
    wv_r = WvT.ap().rearrange("(ko p) n -> p ko n", p=P)
    wo_r = WoT.ap().rearrange("(ho p) n -> p ho n", p=P)   # [128, 2, 2048]

    with tile.TileContext(nc) as tc:
        with (
            tc.tile_pool(name="const", bufs=1) as cp,
            tc.tile_pool(name="data", bufs=1) as dp,
            tc.tile_pool(name="xs", bufs=2) as xp,
            tc.tile_pool(name="work", bufs=3) as wp,
        ):
            wq_sb = cp.tile([P, KD, DLOC], _dt.bfloat16, tag="wq")
            wk_sb = cp.tile([P, KD, DLOC], _dt.bfloat16, tag="wk")
            wv_sb = cp.tile([P, KD, DLOC], _dt.bfloat16, tag="wv")
            wo_sb = cp.tile([P, HL, D], _dt.bfloat16, tag="wo")
            cos_sb = cp.tile([P, T], _dt.bfloat16, tag="cos")
            sin_sb = cp.tile([P, T], _dt.bfloat16, tag="sin")
            idt_sb = cp.tile([P, P], _dt.bfloat16, tag="idt")
            rot_sb = cp.tile([P, P], _dt.bfloat16, tag="rot")
            md_sb = cp.tile([P, 4, 512], _dt.bfloat16, tag="md")

            # persistent activations (partition = head-dim except v_sb);
            # RoPE is applied in place.
            qt_sb = dp.tile([P, HL, TOK], _dt.bfloat16, tag="qt")
            kt_sb = dp.tile([P, HL, TOK], _dt.bfloat16, tag="kt")
            # V in token-major layout with a ones column per head:
            # [128 tokens, token-tile, head, dk | 1]
            v_sb = dp.tile([P, TOK // P, HL, DK + 1], _dt.bfloat16, tag="v")

            # ones column for the fused denominator (written once)
            nc.vector.memset(v_sb[:, :, :, DK], 1.0)

            # ------- phase A: QKV projections with RoPE interleaved -------
            with tc.tile_pool(name="psproj", bufs=1, space="PSUM") as pp, \
                 tc.tile_pool(name="psv", bufs=2, space="PSUM") as pv:
                # batch-1 token tiles first so phase B (which starts with
                # batch 1) never waits on the phase-A tail
                for idx, nt in enumerate([4, 5, 6, 7, 0, 1, 2, 3]):
                    ts0 = nt * 512
                    # one batched 2MB DMA per token tile (HWDGE cost is
                    # dominated by per-instruction overhead)
                    xts = xp.tile([P, KD, 512], _dt.bfloat16, tag="xt")
                    if idx == 0:
                        # chunked first tile + interleaved one-time weight
                        # loads so the first matmuls start within a few us
                        for kc in [(0, 1), (1, 2), (2, 4)] + \
                                  [(k, k + 4) for k in range(4, KD, 4)]:
                            a, z = kc
                            nc.sync.dma_start(xts[:, a:z, :],
                                              xT_r[:, a:z, ts0:ts0 + 512])
                            nc.sync.dma_start(wq_sb[:, a:z, :],
                                              wq_r[:, a:z, :])
                            nc.sync.dma_start(wk_sb[:, a:z, :],
                                              wk_r[:, a:z, :])
                            nc.sync.dma_start(wv_sb[:, a:z, :],
                                              wv_r[:, a:z, :])
                        # must be emitted before their first readers (the
                        # first RoPE) -- dep tracking is program-order
                        nc.sync.dma_start(cos_sb[:], COS[:])
                        nc.sync.dma_start(sin_sb[:], SIN[:])
                        nc.sync.dma_start(idt_sb[:], IDT[:])
                        nc.sync.dma_start(rot_sb[:], ROT[:])
                    else:
                        nc.sync.dma_start(xts[:], xT_r[:, :, ts0:ts0 + 512])
                        if idx == 1:
                            nc.sync.dma_start(md_sb[:], MD[:])
                            nc.sync.dma_start(wo_sb[:], wo_r)
                    psQ = pp.tile([P, HL, 512], _dt.float32, tag="psQ")
                    psK = pp.tile([P, HL, 512], _dt.float32, tag="psK")
                    for k in range(KD):
                        st = (k == 0)
                        sp = (k == KD - 1)
                        for m in range(HL):
                            nc.tensor.matmul(psQ[:, m, :],
                                             wq_sb[:, k, m * P:(m + 1) * P],
                                             xts[:, k, :], start=st, stop=sp)
                            nc.tensor.matmul(psK[:, m, :],
                                             wk_sb[:, k, m * P:(m + 1) * P],
                                             xts[:, k, :], start=st, stop=sp)
                    for m in range(HL):
                        nc.scalar.copy(qt_sb[:, m, ts0:ts0 + 512],
                                       psQ[:, m, :])
                        nc.scalar.copy(kt_sb[:, m, ts0:ts0 + 512],
                                       psK[:, m, :])

                    # V in natural layout: one PSUM bank per token block.
                    # The RoPE rot matmuls (which mix adjacent PARTITIONS,
                    # so they need the PE) are sandwiched between the V
                    # halves so the RoPE DVE chain of the final tile drains
                    # under PE work instead of stalling the phase switch.
                    def emit_v(tb):
                        psv = pv.tile([P, DLOC], _dt.float32, tag="psV",
                                      name=f"psv_{nt}_{tb}")
                        for k in range(KD):
                            nc.tensor.matmul(psv[:],
                                             xts[:, k, tb * P:(tb + 1) * P],
                                             wv_sb[:, k, :],
                                             start=(k == 0), stop=(k == KD - 1))
                        for m in range(HL):
                            nc.scalar.copy(
                                v_sb[:, nt * 4 + tb, m, 0:DK],
                                psv[:, m * P:(m + 1) * P])

                    emit_v(0)
                    emit_v(1)
                    c0 = (nt % QT_PER_B) * 512
                    for src in (qt_sb, kt_sb):
                        for m in range(HL):
                            rp = pv.tile([P, 512], _dt.float32, tag="rot")
                            nc.tensor.matmul(rp[:], rot_sb[:],
                                             src[:, m, ts0:ts0 + 512],
                                             start=True, stop=True)
                            t1 = wp.tile([P, 512], _dt.float32, tag="t1")
                            nc.vector.tensor_mul(t1[:],
                                                 src[:, m, ts0:ts0 + 512],
                                                 cos_sb[:, c0:c0 + 512])
                            t2 = wp.tile([P, 512], _dt.float32, tag="t2")
                            nc.vector.tensor_mul(t2[:], rp[:],
                                                 sin_sb[:, c0:c0 + 512])
                            nc.vector.tensor_add(src[:, m, ts0:ts0 + 512],
                                                 t1[:], t2[:])
                    emit_v(2)
                    emit_v(3)

            # ------- phase B: attention with output proj interleaved -------
            # scores [keys, queries]; exp tiles pT become the STATIONARY
            # operand of P@[V|1], giving op[qc] = [128q, dk | l] per query
            # chunk; normalize on DVE, transpose back on PE.
            # Phase-B software pipeline: each (b,qt) group's output
            # projection is deferred and spliced as PE filler into the NEXT
            # group's exp-paced QK loop, so the in-order PE queue never
            # starves while the ACT engine computes exps.
            with tc.tile_pool(name="psatt", bufs=1, space="PSUM") as pa, \
                 tc.tile_pool(name="psy", bufs=2, space="PSUM") as py:
                filler = []

                def pop_filler(n):
                    for _ in range(min(n, len(filler))):
                        filler.pop(0)()

                for b in (1, 0):
                    # smallest tile first (it gets no filler work), then
                    # descending so later groups keep the filler pipe full
                    for qt in ([0, 3, 2, 1] if b == 1 else [3, 2, 1, 0]):
                        q0 = b * T + qt * 512
                        nk = (qt + 1) * 4
                        otT_sbs = []
                        pend_T = [None]

                        def emit_T(hl, ot_sb, out_list, b_=b, qt_=qt):
                            # lives in the "op" tag rotation (same banks)
                            otT_ps = pa.tile([P, 512], _dt.bfloat16,
                                             tag="op", bufs=2,
                                             padded_shape=[P, 1024],
                                             name=f"oT_{b_}_{qt_}_{hl}")
                            for qc in range(4):
                                nc.tensor.transpose(
                                    otT_ps[:, qc * P:(qc + 1) * P],
                                    ot_sb[:, qc, :], idt_sb)
                            otT = wp.tile([P, 512], _dt.bfloat16, tag="otTs",
                                          bufs=4, name=f"oTs_{b_}_{qt_}_{hl}")
                            nc.vector.tensor_copy(otT, otT_ps)
                            out_list.append(otT)

                        for hl in range(HL):
                            ops = {}
                            ot_sb = wp.tile([P, 4, P], _dt.bfloat16,
                                            tag="ot", bufs=2,
                                            name=f"ot_{b}_{qt}_{hl}")

                            # kt pairs processed DIAGONAL-FIRST: the diag
                            # tiles' exp->mask(DVE)->PV chain warms up at
                            # the loop start; the mask-free off-diagonal
                            # tiles close out the loop with no DVE dep.
                            order = list(range(4 * qt, nk)) + \
                                list(range(4 * qt))
                            prs = [(order[2 * i], order[2 * i + 1])
                                   for i in range(nk // 2)]
                            pos = {kt: i for i, kt in enumerate(order)}
                            pTs = {}

                            def emit_qk_pair(pr):
                                # two key tiles share one PSUM pair tile and
                                # ONE exp instruction (halves ACT overhead);
                                # the unwritten tail of diagonal tiles is
                                # exp'd too but never read downstream.
                                sp_ = pa.tile([P, 2, 512], _dt.float32,
                                              tag="s", bufs=2,
                                              name=f"s_{b}_{qt}_{hl}_{pr}")
                                pT = wp.tile([P, 2, 512], _dt.bfloat16,
                                             tag="pT", bufs=10,
                                             name=f"p_{b}_{qt}_{hl}_{pr}")
                                for half, kt in enumerate(prs[pr]):
                                    j = kt - 4 * qt
                                    qoff = max(j, 0) * P
                                    nq = 512 - qoff
                                    k0 = b * T + kt * P
                                    nc.tensor.matmul(
                                        sp_[:, half, :nq],
                                        kt_sb[:, hl, k0:k0 + P],
                                        qt_sb[:, hl, q0 + qoff:q0 + 512],
                                        start=True, stop=True)
                                nc.scalar.activation(
                                    pT[:], sp_[:],
                                    mybir.ActivationFunctionType.Exp,
                                    scale=SCALE)
                                for half, kt in enumerate(prs[pr]):
                                    j = kt - 4 * qt
                                    qoff = max(j, 0) * P
                                    if j >= 0:  # 0/1 mask in the diagonal
                                        nc.vector.tensor_mul(
                                            pT[:, half, :512 - qoff],
                                            pT[:, half, :512 - qoff],
                                            md_sb[:, j, qoff:])
                                    pTs[kt] = (pT, half, qoff)

                            def emit_pv(kt, qcs):
                                pT, half, qoff = pTs[kt]
                                for qc in qcs:
                                    if kt > 4 * qt + qc:
                                        continue  # fully masked
                                    valid = [k for k in order
                                             if k <= 4 * qt + qc]
                                    st = (kt == valid[0])
                                    sp2 = (kt == valid[-1])
                                    if st:
                                        ops[qc] = pa.tile(
                                            [P, DK + 1], _dt.float32,
                                            tag="op", bufs=2,
                                            padded_shape=[P, 512],
                                            name=f"op_{b}_{qt}_{hl}_{qc}")
                                    c0_ = qc * P - qoff
                                    nc.tensor.matmul(
                                        ops[qc][:, :],
                                        pT[:, half, c0_:c0_ + P],
                                        v_sb[:, b * (T // P) + kt, hl, :],
                                        start=st, stop=sp2)
                                    if sp2:
                                        rec = wp.tile(
                                            [P, 1], _dt.float32, tag="rec",
                                            bufs=8,
                                            name=f"rc_{b}_{qt}_{hl}_{qc}")
                                        nc.vector.reciprocal(
                                            rec, ops[qc][:, DK:DK + 1])
                                        nc.vector.tensor_scalar_mul(
                                            ot_sb[:, qc, :],
                                            ops[qc][:, 0:DK], rec)

                            # pipeline: QK/exp one pair ahead of pass-1
                            # PVs; filler plugs the exp-paced PE slots;
                            # the previous head's transposes are spliced
                            # in after this head's first pair.
                            npr = nk // 2
                            emit_qk_pair(0)
                            for pr in range(1, npr):
                                emit_qk_pair(pr)
                                if pr == 1 and pend_T[0] is not None:
                                    pend_T[0]()
                                    pend_T[0] = None
                                emit_pv(prs[pr - 1][0], (0, 1))
                                emit_pv(prs[pr - 1][1], (0, 1))
                                pop_filler(1)
                            if pend_T[0] is not None:
                                pend_T[0]()
                                pend_T[0] = None
                            emit_pv(prs[npr - 1][0], (0, 1))
                            emit_pv(prs[npr - 1][1], (0, 1))
                            pop_filler(3)
                            for kt in order:
                                emit_pv(kt, (2, 3))
                            pop_filler(2)
                            pend_T[0] = (lambda hl_=hl, ot_=ot_sb,
                                         f_=emit_T, lst_=otT_sbs:
                                         f_(hl_, ot_, lst_))

                        # flush any remaining previous-group output work,
                        # then defer THIS group's output projection as
                        # filler for the next group's exp-paced QK loop.
                        pop_filler(len(filler))
                        last_grp = (b == 0 and qt == 0)
                        filler.append(pend_T[0])
                        pend_T[0] = None
                        state = {}

                        def mk_nb(nb, b_=b, qt_=qt, q0_=q0, otl=otT_sbs,
                                  st_=state, last=last_grp):
                            def go():
                                i = nb % 4
                                if i == 0:
                                    if nb > 0:
                                        nc.sync.dma_start(
                                            yT[(nb - 4) * P:nb * P,
                                               q0_:q0_ + 512]
                                            .rearrange("(i p) q -> p i q",
                                                       p=P), st_["ysb"])
                                    st_["ysb"] = wp.tile(
                                        [P, 4, 512], _dt.bfloat16,
                                        tag="ysb", bufs=3,
                                        name=f"ysb_{b_}_{qt_}_{nb}")
                                yp = py.tile([P, 512], _dt.float32, tag="y",
                                             name=f"y_{b_}_{qt_}_{nb}")
                                for hl in range(HL):
                                    nc.tensor.matmul(
                                        yp,
                                        wo_sb[:, hl, nb * P:(nb + 1) * P],
                                        otl[hl],
                                        start=(hl == 0), stop=(hl == HL - 1))
                                if nb % 3 == 1:
                                    nc.scalar.copy(st_["ysb"][:, i, :], yp)
                                else:
                                    nc.vector.tensor_copy(
                                        st_["ysb"][:, i, :], yp)
                                if last and nb in (13, 15):
                                    # halve the final DMAs so the kernel
                                    # tail is a 2-row transfer, not four
                                    nc.sync.dma_start(
                                        yT[(nb - 1) * P:(nb + 1) * P,
                                           q0_:q0_ + 512]
                                        .rearrange("(i p) q -> p i q", p=P),
                                        st_["ysb"][:, i - 1:i + 1, :])
                                elif nb == 15 and not last:
                                    nc.sync.dma_start(
                                        yT[12 * P:16 * P, q0_:q0_ + 512]
                                        .rearrange("(i p) q -> p i q", p=P),
                                        st_["ysb"])
                            return go

                        for nb in range(16):
                            filler.append(mk_nb(nb))
                # drain the last group's deferred output projection
                pop_filler(len(filler))

    nc.compile()
    return nc


_NC_CACHE = None


def _get_nc():
    global _NC_CACHE
    if _NC_CACHE is None:
        _NC_CACHE = _build_kernel()
    return _NC_CACHE


def _rope_tables():
    inv_freq = 1.0 / THETA ** (np.arange(0, DK, 2, dtype=np.float32) / DK)
    t = np.arange(T, dtype=np.float32)
    freqs = np.outer(t, inv_freq)                 # (T, dk/2)
    freqs = np.repeat(freqs, 2, axis=-1)          # (T, dk)
    return np.cos(freqs), np.sin(freqs)


def _host_inputs(x, Wq, Wk, Wv, Wo):
    """Build the per-core input maps (all host-side prep is free)."""
    xT = np.ascontiguousarray(
        x.reshape(TOK, D).T).astype(BF16)          # [D, B*T]
    cos, sin = _rope_tables()                      # (T, dk)
    cosT = np.ascontiguousarray(cos.T).astype(BF16)  # [128, T]
    sinT = np.ascontiguousarray(sin.T).astype(BF16)

    idt = np.eye(P, dtype=np.float32).astype(BF16)

    rot = np.zeros((P, P), dtype=np.float32)
    for i in range(P // 2):
        rot[2 * i + 1, 2 * i] = -1.0   # (R^T)[2i, 2i+1] = -1
        rot[2 * i, 2 * i + 1] = 1.0    # (R^T)[2i+1, 2i] = +1
    rot = rot.astype(BF16)

    # diagonal-block masks, scores layout [key, query]; offset j*128
    md = np.zeros((4, P, 512), dtype=np.float32)
    kk = np.arange(P)[:, None]
    qq = np.arange(512)[None, :]
    for j in range(4):
        md[j] = (qq >= kk + j * P).astype(np.float32)
    md = np.ascontiguousarray(md.transpose(1, 0, 2)).astype(BF16)

    in_maps = []
    for c in range(NCORES):
        rows = slice(c * DLOC, (c + 1) * DLOC)
        in_maps.append({
            "xT": xT,
            "WqT": np.ascontiguousarray(Wq[rows, :].T).astype(BF16),
            "WkT": np.ascontiguousarray(Wk[rows, :].T).astype(BF16),
            "WvT": np.ascontiguousarray(Wv[rows, :].T).astype(BF16),
            "WoT": np.ascontiguousarray(Wo[:, rows].T).astype(BF16),
            "COS": cosT, "SIN": sinT, "IDT": idt, "ROT": rot, "MD": md,
        })
    return in_maps


def _run(in_maps, **kwargs):
    nc = _get_nc()
    return run_bass_kernel_spmd(nc, in_maps, core_ids=list(range(NCORES)),
                                **kwargs)


def kernel(x, Wq, Wk, Wv, Wo, mask, _bench_results=None, **_kw):
    x = np.asarray(x, dtype=np.float32)
    Wq = np.asarray(Wq, dtype=np.float32)
    Wk = np.asarray(Wk, dtype=np.float32)
    Wv = np.asarray(Wv, dtype=np.float32)
    Wo = np.asarray(Wo, dtype=np.float32)
    mask = np.asarray(mask)
    causal = np.array_equal(mask.reshape(T, T),
                            np.tril(np.ones((T, T), dtype=bool)))
    if not causal:
        raise NotImplementedError("kernel specialized for the causal mask")

    res = _run(_host_inputs(x, Wq, Wk, Wv, Wo))
    if _bench_results is not None:
        _bench_results.append(res)

    acc = np.zeros((D, TOK), dtype=np.float32)
    for r in res.results:
        acc += r["yT"].astype(np.float32)
    # yT[n, b*T + t] -> out[b, t, n]
    return np.ascontiguousarray(acc.reshape(D, B, T).transpose(1, 2, 0))


# revision 34
# speedup vs baseline: 1.0077x; 1.0077x over previous
"""Trainium2 Bass kernel for causal multi-head attention with RoPE.

Reference computation (B=2, T=2048, D=2048, H=16, dk=128):
    Q = x @ Wq.T ; K = x @ Wk.T ; V = x @ Wv.T          (per-head split)
    Q, K <- RoPE(Q, K)
    attn = softmax(mask(Q K^T / sqrt(dk)))
    out  = (attn @ V) merged-heads @ Wo.T
    mask = causal

Sharding (Megatron-style tensor parallel over heads): each of the 8 cores
owns 2 heads (both batches).  Wq/Wk/Wv are sharded column-wise (rows of the
transposed weight), Wo row-wise.  Each core computes a full-shape partial
y^T and the host sums the 8 partials (the all-reduce after Wo).

Device layout choices:
  - x is fed pre-transposed (xT: [D, B*T]) so projections produce Q^T/K^T
    with head-dim on partitions -- the layout QK^T wants.
  - V is produced in natural token-major layout directly by swapping the
    matmul operand roles, with a ones column appended per head so the
    P@[V|1] matmul yields both the attention numerator and the softmax
    denominator in one accumulation (no separate denominator matmul).
  - scores are computed transposed ([keys, queries]); exp(scores) tiles are
    used as the STATIONARY operand of the P@V matmul, so each [128-queries]
    chunk accumulates [q, dk|l] in PSUM; a cheap PE transpose (128 cols per
    query block) restores the [dk, q] layout the output projection needs.
  - no max-subtraction in softmax: scaled scores are ~N(0,1), exp is safe
    in fp32 by a huge margin and matches the reference mathematically.
  - causal masking: off-diagonal key tiles are skipped entirely; the 4
    distinct diagonal-block patterns are multiplicative 0/1 bf16 masks
    applied to exp(scores).
  - RoPE rotate-every-two runs on the DVE with strided even/odd views
    (no PE work); cos/sin muls fused in.
All matmuls run in bf16 (1 cycle/row on the PE vs 4 for fp32).
"""

import sys

sys.path.insert(0, "/opt/trn_rl_repo")

import numpy as np
import ml_dtypes

import concourse.bass as bass  # noqa: F401  (registers engine classes)
import concourse.mybir as mybir
import concourse.tile as tile
from concourse import bacc
from concourse.bass_utils import run_bass_kernel_spmd

BF16 = ml_dtypes.bfloat16

B, T, D, H = 2, 2048, 2048, 16
DK = D // H          # 128
THETA = 10000.0
NCORES = 8
HL = H // NCORES     # 2 local heads per core
DLOC = HL * DK       # 256 local output dims per projection
TOK = B * T          # 4096
P = 128
KD = D // P          # 16 contraction tiles
NT = TOK // 512      # 8 token tiles of 512
QT_PER_B = T // 512  # 4 query tiles per batch
SCALE = 1.0 / float(np.sqrt(DK))

_dt = mybir.dt


def _build_kernel():
    nc = bacc.Bacc("TRN2", target_bir_lowering=False, debug=False,
                   num_devices=NCORES)

    xT = nc.dram_tensor("xT", [D, TOK], _dt.bfloat16, kind="ExternalInput")
    WqT = nc.dram_tensor("WqT", [D, DLOC], _dt.bfloat16, kind="ExternalInput")
    WkT = nc.dram_tensor("WkT", [D, DLOC], _dt.bfloat16, kind="ExternalInput")
    WvT = nc.dram_tensor("WvT", [D, DLOC], _dt.bfloat16, kind="ExternalInput")
    WoT = nc.dram_tensor("WoT", [DLOC, D], _dt.bfloat16, kind="ExternalInput")
    COS = nc.dram_tensor("COS", [P, T], _dt.bfloat16, kind="ExternalInput")
    SIN = nc.dram_tensor("SIN", [P, T], _dt.bfloat16, kind="ExternalInput")
    IDT = nc.dram_tensor("IDT", [P, P], _dt.bfloat16, kind="ExternalInput")
    ROT = nc.dram_tensor("ROT", [P, P], _dt.bfloat16, kind="ExternalInput")
    MD = nc.dram_tensor("MD", [P, 4, 512], _dt.bfloat16, kind="ExternalInput")
    # bf16 partials: halves the output DMA; host accumulates in fp32
    yT = nc.dram_tensor("yT", [D, TOK], _dt.bfloat16, kind="ExternalOutput")

    xT_r = xT.ap().rearrange("(ko p) m -> p ko m", p=P)    # [128, 16, 4096]
    wq_r = WqT.ap().rearrange("(ko p) n -> p ko n", p=P)   # [128, 16, 256]
    wk_r = WkT.ap().rearrange("(ko p) n -> p ko n", p=P)
    wv_r = WvT.ap().rearrange("(ko p) n -> p ko n", p=P)
    wo_r = WoT.ap().rearrange("(ho p) n -> p ho n", p=P)   # [128, 2, 2048]

    with tile.TileContext(nc) as tc:
        with (
            tc.tile_pool(name="const", bufs=1) as cp,
            tc.tile_pool(name="data", bufs=1) as dp,
            tc.tile_pool(name="xs", bufs=2) as xp,
            tc.tile_pool(name="work", bufs=3) as wp,
        ):
            wq_sb = cp.tile([P, KD, DLOC], _dt.bfloat16, tag="wq")
            wk_sb = cp.tile([P, KD, DLOC], _dt.bfloat16, tag="wk")
            wv_sb = cp.tile([P, KD, DLOC], _dt.bfloat16, tag="wv")
            wo_sb = cp.tile([P, HL, D], _dt.bfloat16, tag="wo")
            cos_sb = cp.tile([P, T], _dt.bfloat16, tag="cos")
            sin_sb = cp.tile([P, T], _dt.bfloat16, tag="sin")
            idt_sb = cp.tile([P, P], _dt.bfloat16, tag="idt")
            rot_sb = cp.tile([P, P], _dt.bfloat16, tag="rot")
            md_sb = cp.tile([P, 4, 512], _dt.bfloat16, tag="md")

            # persistent activations (partition = head-dim except v_sb);
            # RoPE is applied in place.
            qt_sb = dp.tile([P, HL, TOK], _dt.bfloat16, tag="qt")
            kt_sb = dp.tile([P, HL, TOK], _dt.bfloat16, tag="kt")
            # V in token-major layout with a ones column per head:
            # [128 tokens, token-tile, head, dk | 1]
            v_sb = dp.tile([P, TOK // P, HL, DK + 1], _dt.bfloat16, tag="v")

            # ones column for the fused denominator (written once)
            nc.vector.memset(v_sb[:, :, :, DK], 1.0)

            # ------- phase A: QKV projections with RoPE interleaved -------
            with tc.tile_pool(name="psproj", bufs=1, space="PSUM") as pp, \
                 tc.tile_pool(name="psv", bufs=2, space="PSUM") as pv:
                # batch-1 token tiles first so phase B (which starts with
                # batch 1) never waits on the phase-A tail
                for idx, nt in enumerate([4, 5, 6, 7, 0, 1, 2, 3]):
                    ts0 = nt * 512
                    # one batched 2MB DMA per token tile (HWDGE cost is
                    # dominated by per-instruction overhead)
                    xts = xp.tile([P, KD, 512], _dt.bfloat16, tag="xt")
                    if idx == 0:
                        # chunked first tile + interleaved one-time weight
                        # loads so the first matmuls start within a few us
                        for kc in [(0, 1), (1, 2), (2, 4)] + \
                                  [(k, k + 4) for k in range(4, KD, 4)]:
                            a, z = kc
                            nc.sync.dma_start(xts[:, a:z, :],
                                              xT_r[:, a:z, ts0:ts0 + 512])
                            nc.sync.dma_start(wq_sb[:, a:z, :],
                                              wq_r[:, a:z, :])
                            nc.sync.dma_start(wk_sb[:, a:z, :],
                                              wk_r[:, a:z, :])
                            nc.sync.dma_start(wv_sb[:, a:z, :],
                                              wv_r[:, a:z, :])
                        # must be emitted before their first readers (the
                        # first RoPE) -- dep tracking is program-order
                        nc.sync.dma_start(cos_sb[:], COS[:])
                        nc.sync.dma_start(sin_sb[:], SIN[:])
                        nc.sync.dma_start(idt_sb[:], IDT[:])
                        nc.sync.dma_start(rot_sb[:], ROT[:])
                    else:
                        nc.sync.dma_start(xts[:], xT_r[:, :, ts0:ts0 + 512])
                        if idx == 1:
                            nc.sync.dma_start(md_sb[:], MD[:])
                            nc.sync.dma_start(wo_sb[:], wo_r)
                    psQ = pp.tile([P, HL, 512], _dt.float32, tag="psQ")
                    psK = pp.tile([P, HL, 512], _dt.float32, tag="psK")
                    for k in range(KD):
                        st = (k == 0)
                        sp = (k == KD - 1)
                        for m in range(HL):
                            nc.tensor.matmul(psQ[:, m, :],
                                             wq_sb[:, k, m * P:(m + 1) * P],
                                             xts[:, k, :], start=st, stop=sp)
                            nc.tensor.matmul(psK[:, m, :],
                                             wk_sb[:, k, m * P:(m + 1) * P],
                                             xts[:, k, :], start=st, stop=sp)
                    for m in range(HL):
                        nc.scalar.copy(qt_sb[:, m, ts0:ts0 + 512],
                                       psQ[:, m, :])
                        nc.scalar.copy(kt_sb[:, m, ts0:ts0 + 512],
                                       psK[:, m, :])

                    # V in natural layout: one PSUM bank per token block.
                    # The RoPE rot matmuls (which mix adjacent PARTITIONS,
                    # so they need the PE) are sandwiched between the V
                    # halves so the RoPE DVE chain of the final tile drains
                    # under PE work instead of stalling the phase switch.
                    def emit_v(tb):
                        psv = pv.tile([P, DLOC], _dt.float32, tag="psV",
                                      name=f"psv_{nt}_{tb}")
                        for k in range(KD):
                            nc.tensor.matmul(psv[:],
                                             xts[:, k, tb * P:(tb + 1) * P],
                                             wv_sb[:, k, :],
                                             start=(k == 0), stop=(k == KD - 1))
                        for m in range(HL):
                            nc.scalar.copy(
                                v_sb[:, nt * 4 + tb, m, 0:DK],
                                psv[:, m * P:(m + 1) * P])

                    emit_v(0)
                    emit_v(1)
                    c0 = (nt % QT_PER_B) * 512
                    for src in (qt_sb, kt_sb):
                        for m in range(HL):
                            rp = pv.tile([P, 512], _dt.float32, tag="rot")
                            nc.tensor.matmul(rp[:], rot_sb[:],
                                             src[:, m, ts0:ts0 + 512],
                                             start=True, stop=True)
                            t1 = wp.tile([P, 512], _dt.float32, tag="t1")
                            nc.vector.tensor_mul(t1[:],
                                                 src[:, m, ts0:ts0 + 512],
                                                 cos_sb[:, c0:c0 + 512])
                            t2 = wp.tile([P, 512], _dt.float32, tag="t2")
                            nc.vector.tensor_mul(t2[:], rp[:],
                                                 sin_sb[:, c0:c0 + 512])
                            nc.vector.tensor_add(src[:, m, ts0:ts0 + 512],
                                                 t1[:], t2[:])
                    emit_v(2)
                    emit_v(3)

            # ------- phase B: attention with output proj interleaved -------
            # scores [keys, queries]; exp tiles pT become the STATIONARY
            # operand of P@[V|1], giving op[qc] = [128q, dk | l] per query
            # chunk; normalize on DVE, transpose back on PE.
            # Phase-B software pipeline: each (b,qt) group's output
            # projection is deferred and spliced as PE filler into the NEXT
            # group's exp-paced QK loop, so the in-order PE queue never
            # starves while the ACT engine computes exps.
            with tc.tile_pool(name="psatt", bufs=1, space="PSUM") as pa, \
                 tc.tile_pool(name="psy", bufs=2, space="PSUM") as py:
                filler = []

                def pop_filler(n):
                    for _ in range(min(n, len(filler))):
                        filler.pop(0)()

                for b in (1, 0):
                    # smallest tile first (it gets no filler work), then
                    # descending so later groups keep the filler pipe full
                    for qt in ([0, 3, 2, 1] if b == 1 else [3, 2, 1, 0]):
                        q0 = b * T + qt * 512
                        nk = (qt + 1) * 4
                        otT_sbs = []
                        pend_T = [None]

                        def emit_T(hl, ot_sb, out_list, b_=b, qt_=qt):
                            # lives in the "op" tag rotation (same banks)
                            otT_ps = pa.tile([P, 512], _dt.bfloat16,
                                             tag="op", bufs=2,
                                             padded_shape=[P, 1024],
                                             name=f"oT_{b_}_{qt_}_{hl}")
                            for qc in range(4):
                                nc.tensor.transpose(
                                    otT_ps[:, qc * P:(qc + 1) * P],
                                    ot_sb[:, qc, :], idt_sb)
                            otT = wp.tile([P, 512], _dt.bfloat16, tag="otTs",
                                          bufs=4, name=f"oTs_{b_}_{qt_}_{hl}")
                            nc.vector.tensor_copy(otT, otT_ps)
                            out_list.append(otT)

                        for hl in range(HL):
                            ops = {}
                            ot_sb = wp.tile([P, 4, P], _dt.bfloat16,
                                            tag="ot", bufs=2,
                                            name=f"ot_{b}_{qt}_{hl}")

                            order = list(range(nk))
                            prs = [(order[2 * i], order[2 * i + 1])
                                   for i in range(nk // 2)]
                            pTs = {}

                            def emit_qk_pair(pr):
                                # two key tiles share one PSUM pair tile and
                                # ONE exp instruction (halves ACT overhead);
                                # the unwritten tail of diagonal tiles is
                                # exp'd too but never read downstream.
                                sp_ = pa.tile([P, 2, 512], _dt.float32,
                                              tag="s", bufs=2,
                                              name=f"s_{b}_{qt}_{hl}_{pr}")
                                pT = wp.tile([P, 2, 512], _dt.bfloat16,
                                             tag="pT", bufs=10,
                                             name=f"p_{b}_{qt}_{hl}_{pr}")
                                for half, kt in enumerate(prs[pr]):
                                    j = kt - 4 * qt
                                    qoff = max(j, 0) * P
                                    nq = 512 - qoff
                                    k0 = b * T + kt * P
                                    nc.tensor.matmul(
                                        sp_[:, half, :nq],
                                        kt_sb[:, hl, k0:k0 + P],
                                        qt_sb[:, hl, q0 + qoff:q0 + 512],
                                        start=True, stop=True)
                                nc.scalar.activation(
                                    pT[:], sp_[:],
                                    mybir.ActivationFunctionType.Exp,
                                    scale=SCALE)
                                for half, kt in enumerate(prs[pr]):
                                    j = kt - 4 * qt
                                    qoff = max(j, 0) * P
                                    if j >= 0:  # 0/1 mask in the diagonal
                                        nc.vector.tensor_mul(
                                            pT[:, half, :512 - qoff],
                                            pT[:, half, :512 - qoff],
                                            md_sb[:, j, qoff:])
                                    pTs[kt] = (pT, half, qoff)

                            def emit_pv(kt, qcs):
                                pT, half, qoff = pTs[kt]
                                for qc in qcs:
                                    if kt > 4 * qt + qc:
                                        continue  # fully masked
                                    valid = [k for k in order
                                             if k <= 4 * qt + qc]
                                    st = (kt == valid[0])
                                    sp2 = (kt == valid[-1])
                                    if st:
                                        ops[qc] = pa.tile(
                                            [P, DK + 1], _dt.float32,
                                            tag="op", bufs=2,
                                            padded_shape=[P, 512],
                                            name=f"op_{b}_{qt}_{hl}_{qc}")
                                    c0_ = qc * P - qoff
                                    nc.tensor.matmul(
                                        ops[qc][:, :],
                                        pT[:, half, c0_:c0_ + P],
                                        v_sb[:, b * (T // P) + kt, hl, :],
                                        start=st, stop=sp2)
                                    if sp2:
                                        rec = wp.tile(
                                            [P, 1], _dt.float32, tag="rec",
                                            bufs=8,
                                            name=f"rc_{b}_{qt}_{hl}_{qc}")
                                        nc.vector.reciprocal(
                                            rec, ops[qc][:, DK:DK + 1])
                                        nc.vector.tensor_scalar_mul(
                                            ot_sb[:, qc, :],
                                            ops[qc][:, 0:DK], rec)

                            # pipeline: QK/exp one pair ahead of pass-1
                            # PVs; filler plugs the exp-paced PE slots;
                            # the previous head's transposes are spliced
                            # in after this head's first pair.
                            npr = nk // 2
                            emit_qk_pair(0)
                            for pr in range(1, npr):
                                emit_qk_pair(pr)
                                if pr == 1 and pend_T[0] is not None:
                                    pend_T[0]()
                                    pend_T[0] = None
                                emit_pv(prs[pr - 1][0], (0, 1))
                                emit_pv(prs[pr - 1][1], (0, 1))
                                pop_filler(1)
                            if pend_T[0] is not None:
                                pend_T[0]()
                                pend_T[0] = None
                            emit_pv(prs[npr - 1][0], (0, 1))
                            emit_pv(prs[npr - 1][1], (0, 1))
                            pop_filler(3)
                            for kt in order:
                                emit_pv(kt, (2, 3))
                            pop_filler(2)
                            pend_T[0] = (lambda hl_=hl, ot_=ot_sb,
                                         f_=emit_T, lst_=otT_sbs:
                                         f_(hl_, ot_, lst_))

                        # flush any remaining previous-group output work,
                        # then defer THIS group's output projection as
                        # filler for the next group's exp-paced QK loop.
                        pop_filler(len(filler))
                        last_grp = (b == 0 and qt == 0)
                        filler.append(pend_T[0])
                        pend_T[0] = None
                        state = {}

                        def mk_nb(nb, b_=b, qt_=qt, q0_=q0, otl=otT_sbs,
                                  st_=state, last=last_grp):
                            def go():
                                i = nb % 4
                                if i == 0:
                                    if nb > 0:
                                        nc.sync.dma_start(
                                            yT[(nb - 4) * P:nb * P,
                                               q0_:q0_ + 512]
                                            .rearrange("(i p) q -> p i q",
                                                       p=P), st_["ysb"])
                                    st_["ysb"] = wp.tile(
                                        [P, 4, 512], _dt.bfloat16,
                                        tag="ysb", bufs=3,
                                        name=f"ysb_{b_}_{qt_}_{nb}")
                                yp = py.tile([P, 512], _dt.float32, tag="y",
                                             name=f"y_{b_}_{qt_}_{nb}")
                                for hl in range(HL):
                                    nc.tensor.matmul(
                                        yp,
                                        wo_sb[:, hl, nb * P:(nb + 1) * P],
                                        otl[hl],
                                        start=(hl == 0), stop=(hl == HL - 1))
                                if nb % 3 == 1:
                                    nc.scalar.copy(st_["ysb"][:, i, :], yp)
                                else:
                                    nc.vector.tensor_copy(
                                        st_["ysb"][:, i, :], yp)
                                if last and nb in (13, 15):
                                    # halve the final DMAs so the kernel
                                    # tail is a 2-row transfer, not four
                                    nc.sync.dma_start(
                                        yT[(nb - 1) * P:(nb + 1) * P,
                                           q0_:q0_ + 512]
                                        .rearrange("(i p) q -> p i q", p=P),
                                        st_["ysb"][:, i - 1:i + 1, :])
                                elif nb == 15 and not last:
                                    nc.sync.dma_start(
                                        yT[12 * P:16 * P, q0_:q0_ + 512]
                                        .rearrange("(i p) q -> p i q", p=P),
                                        st_["ysb"])
                            return go

                        for nb in range(16):
                            filler.append(mk_nb(nb))
                # drain the last group's deferred output projection
                pop_filler(len(filler))

    nc.compile()
    return nc


_NC_CACHE = None


def _get_nc():
    global _NC_CACHE
    if _NC_CACHE is None:
        _NC_CACHE = _build_kernel()
    return _NC_CACHE


def _rope_tables():
    inv_freq = 1.0 / THETA ** (np.arange(0, DK, 2, dtype=np.float32) / DK)
    t = np.arange(T, dtype=np.float32)
    freqs = np.outer(t, inv_freq)                 # (T, dk/2)
    freqs = np.repeat(freqs, 2, axis=-1)          # (T, dk)
    return np.cos(freqs), np.sin(freqs)


def _host_inputs(x, Wq, Wk, Wv, Wo):
    """Build the per-core input maps (all host-side prep is free)."""
    xT = np.ascontiguousarray(
        x.reshape(TOK, D).T).astype(BF16)          # [D, B*T]
    cos, sin = _rope_tables()                      # (T, dk)
    cosT = np.ascontiguousarray(cos.T).astype(BF16)  # [128, T]
    sinT = np.ascontiguousarray(sin.T).astype(BF16)

    idt = np.eye(P, dtype=np.float32).astype(BF16)

    rot = np.zeros((P, P), dtype=np.float32)
    for i in range(P // 2):
        rot[2 * i + 1, 2 * i] = -1.0   # (R^T)[2i, 2i+1] = -1
        rot[2 * i, 2 * i + 1] = 1.0    # (R^T)[2i+1, 2i] = +1
    rot = rot.astype(BF16)

    # diagonal-block masks, scores layout [key, query]; offset j*128
    md = np.zeros((4, P, 512), dtype=np.float32)
    kk = np.arange(P)[:, None]
    qq = np.arange(512)[None, :]
    for j in range(4):
        md[j] = (qq >= kk + j * P).astype(np.float32)
    md = np.ascontiguousarray(md.transpose(1, 0, 2)).astype(BF16)

    in_maps = []
    for c in range(NCORES):
        rows = slice(c * DLOC, (c + 1) * DLOC)
        in_maps.append({
            "xT": xT,
            "WqT": np.ascontiguousarray(Wq[rows, :].T).astype(BF16),
            "WkT": np.ascontiguousarray(Wk[rows, :].T).astype(BF16),
            "WvT": np.ascontiguousarray(Wv[rows, :].T).astype(BF16),
            "WoT": np.ascontiguousarray(Wo[:, rows].T).astype(BF16),
            "COS": cosT, "SIN": sinT, "IDT": idt, "ROT": rot, "MD": md,
        })
    return in_maps


def _run(in_maps, **kwargs):
    nc = _get_nc()
    return run_bass_kernel_spmd(nc, in_maps, core_ids=list(range(NCORES)),
                                **kwargs)


def kernel(x, Wq, Wk, Wv, Wo, mask, _bench_results=None, **_kw):
    x = np.asarray(x, dtype=np.float32)
    Wq = np.asarray(Wq, dtype=np.float32)
    Wk = np.asarray(Wk, dtype=np.float32)
    Wv = np.asarray(Wv, dtype=np.float32)
    Wo = np.asarray(Wo, dtype=np.float32)
    mask = np.asarray(mask)
    causal = np.array_equal(mask.reshape(T, T),
                            np.tril(np.ones((T, T), dtype=bool)))
    if not causal:
        raise NotImplementedError("kernel specialized for the causal mask")

    res = _run(_host_inputs(x, Wq, Wk, Wv, Wo))
    if _bench_results is not None:
        _bench_results.append(res)

    acc = np.zeros((D, TOK), dtype=np.float32)
    for r in res.results:
        acc += r["yT"].astype(np.float32)
    # yT[n, b*T + t] -> out[b, t, n]
    return np.ascontiguousarray(acc.reshape(D, B, T).transpose(1, 2, 0))
